# revision 1
# baseline (speedup 1.0000x reference)
"""Trainium2 Bass kernel for nn_Attention_2851858284976.

Dense transformer attention block, b=8 n=1024 dim=1024 heads=16.
Sharding: pure data parallel — one batch element per NeuronCore (8 cores).

Per-core math (batch element x of shape (n, dim)):
  Y = x @ w_qkv^T                              (n, 3*dim)
  Z = Y.reshape(49152, 64)   # raw reshape: rows are (token, col-block) pairs
  Q = Z[0:16384], K = Z[16384:32768], V = Z[32768:49152], each (16, 1024, 64)
  per head: P^T = exp(scale * K_h @ Q_h^T)     (softmax along the partition axis)
            [O^T; Zs*64] = [V_h | 1*64]^T @ P^T  (ones cols replicate the denom)
            oT_h = O^T * (1/Zs)
  out = (oT stacked).T @ w_out^T + b_out

Phase 1 runs in TWO orientations to avoid any DRAM round-trip for Q/K:
  A-part: Zt[e, i] = Y[i, e] computed directly (lhsT = w_qkv^T stationary,
    rhs = x^T moving) for tokens 0:768, which covers every Q/K Z-row
    (r = 48*i + c0 < 32768 needs i <= 682). Each PSUM tile (128 e-rows x
    384 tokens) is scattered straight into QKall[d, r] with strided
    ACT/DVE copies (f32->bf16) — no stores, no xbar transposes.
  B-part: classic orientation (lhsT = x^T, rhs = w_qkv^T) for tokens
    640:1024, producing the V region, stored packed/contiguous into a
    64-wide v_buf in DRAM and read back per-head during phase 2.

Matmul datapath is bf16 (1 cyc/row on the PE); accumulation fp32 in PSUM.
"""
import numpy as np
import ml_dtypes

import concourse.bass as bass
import concourse.mybir as mybir
from concourse import bacc
from concourse.tile import TileContext
from concourse.bass_utils import run_bass_kernel_spmd

N_CORES = 8
N = 1024          # tokens
DIM = 1024
E3 = 3 * DIM      # qkv projection width
H = 16            # heads
HD = 64           # head dim
SCALE = HD ** -0.5
QKFLAT = 48 * 683             # 32784: Q/K Z-rows 0:32768 plus 16 slack
VROWS = 48 * 342              # 16416: V Z-rows (16384) plus 32 slack

F32 = mybir.dt.float32
BF = mybir.dt.bfloat16
FT = mybir.ActivationFunctionType


def build():
    nc = bacc.Bacc("TRN2", target_bir_lowering=False, num_devices=N_CORES)
    xt = nc.declare_dram_parameter("xt", [DIM, N], BF, isOutput=False)
    wqkvt = nc.declare_dram_parameter("wqkvt", [DIM, E3], BF, isOutput=False)
    woutt = nc.declare_dram_parameter("woutt", [DIM, DIM], BF, isOutput=False)
    bias = nc.declare_dram_parameter("bias", [1, DIM], F32, isOutput=False)
    outp = nc.declare_dram_parameter("out", [N, DIM], F32, isOutput=True)

    with TileContext(nc) as tc:
        with tc.tile_pool(name="dram", bufs=1, space="DRAM") as dpool, \
             tc.tile_pool(name="singles", bufs=1) as singles:
            # packed V buffer: flat row q = Z row 32768+q = v of head q//1024
            vbuf = dpool.tile([VROWS, HD], BF)
            vb3 = vbuf.rearrange("(a c) d -> a c d", c=48)   # (342, 48, 64)

            oT = singles.tile([128, 8, N], BF)    # [64*(h%2)+dd, h//2, i]
            biasrep = singles.tile([128, DIM], F32)
            ones_f = singles.tile([128, 8, HD], F32)
            vh0 = singles.tile([128, 8, 2 * HD], BF)
            vh1 = singles.tile([128, 8, 2 * HD], BF)
            vh2 = singles.tile([128, 8, 2 * HD], BF)
            vh3 = singles.tile([128, 8, 2 * HD], BF)
            vhs = [vh0, vh1, vh2, vh3]

            with tc.tile_pool(name="qk", bufs=1) as qkpool:
                # QKall[d, r] = Z[r, d] for Z rows r < 32768 (Q then K).
                QKall = qkpool.tile([64, QKFLAT], BF)
                # view with the (i, c0) structure: flat r = 48*i + c0
                QKci = QKall.rearrange("p (i c) -> p c i", c=48)  # (64,48,683)

                with tc.tile_pool(name="p1", bufs=1) as p1:
                    XT = p1.tile([128, 8, N], BF)
                    WT = p1.tile([128, 8, E3], BF)
                    # Load order matters: the first A psum group needs ALL
                    # kt slices of XT[:, :, 0:683] and WT[:, :, 0:128], so
                    # bring a 512-wide WT slab for every kt before the bulk.
                    for kt in range(8):
                        nc.sync.dma_start(
                            out=XT[:, kt, :], in_=xt[kt * 128:(kt + 1) * 128, :])
                    for kt in range(8):
                        nc.sync.dma_start(
                            out=WT[:, kt, 0:512],
                            in_=wqkvt[kt * 128:(kt + 1) * 128, 0:512])
                    for kt in range(8):
                        nc.sync.dma_start(
                            out=WT[:, kt, 512:E3],
                            in_=wqkvt[kt * 128:(kt + 1) * 128, 512:E3])
                    nc.sync.dma_start(out=biasrep,
                                      in_=bias[:].to_broadcast((128, DIM)))
                    # [V | ones*64] stationaries for the PV matmul; ones half
                    # replicates the softmax denominator on out rows 64-127.
                    nc.vector.memset(ones_f, 1.0)
                    for v in vhs:
                        nc.vector.tensor_copy(v[:, :, HD:2 * HD], ones_f)
                        nc.vector.tensor_copy(v[:, :, 0:HD], ones_f)

                    # ---------- phase 1A: Zt = w_qkv @ x^T -> QKall --------
                    # One 683-wide psum tile per e-block (both matmul column
                    # groups), then one strided scatter per c0 column. The
                    # scatters are slow (2B writes at 96B stride), so they
                    # rotate across ACT, DVE and GpSimd to stay off the
                    # critical path.
                    def emit_b_tile(bit, ec, pBpool, psBpool, engsel):
                        ps = psBpool.tile([128, 512], F32)
                        for kt in range(8):
                            nc.tensor.matmul(
                                ps,
                                lhsT=XT[:, kt, bit * 128:(bit + 1) * 128],
                                rhs=WT[:, kt, ec * 512:(ec + 1) * 512],
                                start=(kt == 0), stop=(kt == 7))
                        st = pBpool.tile([128, 8, HD], BF)
                        if engsel == 0:
                            nc.scalar.copy(
                                st, ps.rearrange("p (b d) -> p b d", d=HD))
                        else:
                            nc.vector.tensor_copy(
                                st, ps.rearrange("p (b d) -> p b d", d=HD))
                        # V rows: q = 48*T + c0 - 32768
                        #   c0 <= 31: q = 48*(T-683)+(c0+16)
                        #   c0 >= 32: q = 48*(T-682)+(c0-32)
                        if ec <= 3:
                            plo = 43 if bit == 5 else 0
                            nc.sync.dma_start(
                                out=vb3[bit * 128 + plo - 683:
                                        (bit + 1) * 128 - 683,
                                        ec * 8 + 16: ec * 8 + 24, :],
                                in_=st[plo:128, :, :])
                        else:
                            plo = 42 if bit == 5 else 0
                            nc.sync.dma_start(
                                out=vb3[bit * 128 + plo - 682:
                                        (bit + 1) * 128 - 682,
                                        (ec - 4) * 8: (ec - 4) * 8 + 8, :],
                                in_=st[plo:128, :, :])

                    # The strided gathers (1.6us) outpace each A tile's PE
                    # work (1.28us), so B's it5/it6 tiles interleave into the
                    # A sweep to keep the PE fed while the gathers drain.
                    bsched = [(5, e) for e in range(6)] + [(6, e) for e in range(6)]
                    bi = 0
                    a_idx = 0
                    with tc.tile_pool(name="psA", bufs=4, space="PSUM") as psA:
                        for tb in range(2):           # token halves of 384
                            i_lo = tb * 384
                            i_hi = min((tb + 1) * 384, 683)
                            cnt = i_hi - i_lo
                            for eb in range(24):      # e-blocks of 128
                                ps = psA.tile([128, 384], F32)
                                for kt in range(8):
                                    nc.tensor.matmul(
                                        ps[:, 0:cnt],
                                        lhsT=WT[:, kt,
                                                eb * 128:(eb + 1) * 128],
                                        rhs=XT[:, kt, i_lo:i_hi],
                                        start=(kt == 0), stop=(kt == 7))
                                c0 = 2 * eb
                                nc.scalar.copy(
                                    QKci[0:64, c0, i_lo:i_hi],
                                    ps[0:64, 0:cnt])
                                nc.vector.tensor_copy(
                                    QKci[0:64, c0 + 1, i_lo:i_hi],
                                    ps[64:128, 0:cnt])


                    def qt_sl(h, lo, sz):
                        return QKall[0:64, h * N + lo: h * N + lo + sz]

                    def kt_sl(h, lo, sz):
                        return QKall[0:64, 16384 + h * N + lo: 16384 + h * N + lo + sz]

                    with tc.tile_pool(name="p3", bufs=1) as p3:
                        WOT = p3.tile([128, 8, DIM], BF)
                        nc.sync.dma_start(
                            out=WOT, in_=woutt[:].rearrange("(a p) e -> p a e", p=128))

                        # ---------- phase 2 pools (sps coexists with psB) -----
                        with tc.tile_pool(name="pt", bufs=9) as ptpool, \
                             tc.tile_pool(name="rz", bufs=2) as rzpool, \
                             tc.tile_pool(name="sps", bufs=2, space="PSUM") as spsum:
                            steps = [(h, jt) for h in range(H) for jt in range(8)]

                            def load_v(h):
                                nc.sync.dma_start(
                                    out=vhs[h % 4][:, :, 0:HD],
                                    in_=vbuf[h * N:(h + 1) * N, :].rearrange(
                                        "(t p) d -> p t d", p=128))

                            def produce(h, jt):
                                sps = spsum.tile([128, 2, 512], F32, tag="sps")
                                warm = vhs[(h + 2) % 4]
                                # HAM warm-keeper; overwritten by the real scores
                                # matmul (start=True).
                                nc.tensor.matmul(
                                    sps[0:128, 0, 0:128],
                                    lhsT=warm[:, 0, :], rhs=warm[:, 0, :],
                                    start=True, stop=True)
                                for ic in range(2):
                                    nc.tensor.matmul(
                                        sps[:, ic, :],
                                        lhsT=kt_sl(h, jt * 128, 128),
                                        rhs=qt_sl(h, ic * 512, 512),
                                        start=True, stop=True)
                                pt = ptpool.tile([128, 2, 512], BF, tag="pt")
                                nc.scalar.activation(pt, sps, FT.Exp, scale=SCALE)
                                return pt

                            # ------ phase 1B: V region (tokens 640:1024) ------
                            # Head 0's produce steps are emitted between B-it6
                            # and B-it7 so the ACT exp stream starts ~10us
                            # before the PE finishes phase 1.
                            pre_pts = []
                            with tc.tile_pool(name="pB", bufs=6) as pB, \
                                 tc.tile_pool(name="psB", bufs=4,
                                              space="PSUM") as psB:
                                for bit in (5, 6, 7):
                                    if bit == 7:
                                        # pre-produce 2 steps: ACT starts its
                                        # exp stream while B-it7 runs on PE.
                                        pre_pts = [produce(0, jt)
                                                   for jt in range(2)]
                                    for ec in range(6):
                                        emit_b_tile(bit, ec, pB, psB, 1)
                                    if bit == 5:
                                        # V heads 0-3 live entirely in it5's
                                        # rows: preload them now.
                                        for hv in range(4):
                                            nc.sync.dma_start(
                                                out=vhs[hv][:, :, 0:HD],
                                                in_=vbuf[hv * N:(hv + 1) * N,
                                                         :].rearrange(
                                                             "(t p) d -> p t d",
                                                             p=128))

                            # ---------- phase 2: consume + produce-ahead ------
                            with tc.tile_pool(name="ops", bufs=1,
                                              space="PSUM") as opsum:
                                pts = list(pre_pts)
                                ops = None
                                for s, (h, jt) in enumerate(steps):
                                    po, hf = 64 * (h % 2), h // 2
                                    if jt == 0:
                                        pn = h % 2
                                        ops0 = opsum.tile([128, 512], F32,
                                                          tag=f"ops{2 * pn}")
                                        ops1 = opsum.tile([128, 512], F32,
                                                          tag=f"ops{2 * pn + 1}")
                                        ops = (ops0, ops1)
                                    pt_cur = pts.pop(0)
                                    if s + 2 < len(steps):
                                        pts.append(produce(*steps[s + 2]))
                                    for ic in range(2):
                                        nc.tensor.matmul(
                                            ops[ic],
                                            lhsT=vhs[h % 4][:, jt, :],
                                            rhs=pt_cur[:, ic, :],
                                            start=(jt == 0), stop=(jt == 7),
                                            skip_group_check=True)
                                    if jt == 7 and h + 4 < H:
                                        load_v(h + 4)
                                    if jt == 7:
                                        for ic in range(2):
                                            # custom-DVE reciprocal can't read
                                            # PSUM; stage through SBUF.
                                            zst = rzpool.tile([64, 512], F32,
                                                              tag="zst")
                                            nc.vector.tensor_copy(
                                                zst, ops[ic][64:128, :])
                                            rzs = rzpool.tile([64, 512], F32,
                                                              tag="rzs")
                                            nc.vector.reciprocal_approx_fast(
                                                rzs, zst)
                                            nc.vector.tensor_mul(
                                                oT[po:po + 64, hf,
                                                   ic * 512:(ic + 1) * 512],
                                                ops[ic][0:64, :], rzs)

                        # ---------- phase 3: out = oT.T @ w_out^T + b ----------
                        with tc.tile_pool(name="p3st", bufs=4) as p3st, \
                             tc.tile_pool(name="ps3", bufs=4, space="PSUM") as ps3:
                            for it in range(8):
                                for ec in range(2):
                                    rps = ps3.tile([128, 512], F32)
                                    for ct in range(8):
                                        nc.tensor.matmul(
                                            rps,
                                            lhsT=oT[:, ct, it * 128:(it + 1) * 128],
                                            rhs=WOT[:, ct, ec * 512:(ec + 1) * 512],
                                            start=(ct == 0), stop=(ct == 7))
                                    ost = p3st.tile([128, 512], F32)
                                    nc.vector.tensor_add(
                                        ost, rps, biasrep[:, ec * 512:(ec + 1) * 512])
                                    nc.sync.dma_start(
                                        out=outp[it * 128:(it + 1) * 128,
                                                 ec * 512:(ec + 1) * 512],
                                        in_=ost)

    nc.finalize()
    return nc


_CACHE = {}


def _get_nc():
    if "nc" not in _CACHE:
        _CACHE["nc"] = build()
    return _CACHE["nc"]


def make_in_maps(x, w_qkv, w_out, b_out):
    bf = ml_dtypes.bfloat16
    wqkvt = np.ascontiguousarray(np.asarray(w_qkv, dtype=np.float32).T).astype(bf)
    woutt = np.ascontiguousarray(np.asarray(w_out, dtype=np.float32).T).astype(bf)
    bias = np.ascontiguousarray(np.asarray(b_out, dtype=np.float32).reshape(1, DIM))
    x = np.asarray(x, dtype=np.float32)
    return [
        {
            "xt": np.ascontiguousarray(x[b].T).astype(bf),
            "wqkvt": wqkvt,
            "woutt": woutt,
            "bias": bias,
        }
        for b in range(N_CORES)
    ]


def kernel(x, w_qkv, w_out, b_out):
    nc = _get_nc()
    in_maps = make_in_maps(x, w_qkv, w_out, b_out)
    res = run_bass_kernel_spmd(nc, in_maps, core_ids=list(range(N_CORES)))
    return np.stack(
        [res.results[b]["out"] for b in range(N_CORES)], axis=0
    ).astype(np.float32)



# revision 12
# speedup vs baseline: 1.0146x; 1.0146x over previous
"""Trainium2 Bass kernel for nn_Attention_2851858284976.

Dense transformer attention block, b=8 n=1024 dim=1024 heads=16.
Sharding: pure data parallel — one batch element per NeuronCore (8 cores).

Per-core math (batch element x of shape (n, dim)):
  Y = x @ w_qkv^T                              (n, 3*dim)
  Z = Y.reshape(49152, 64)   # raw reshape: rows are (token, col-block) pairs
  Q = Z[0:16384], K = Z[16384:32768], V = Z[32768:49152], each (16, 1024, 64)
  per head: P^T = exp(scale * K_h @ Q_h^T)     (softmax along the partition axis)
            [O^T; Zs*64] = [V_h | 1*64]^T @ P^T  (ones cols replicate the denom)
            oT_h = O^T * (1/Zs)
  out = (oT stacked).T @ w_out^T + b_out

Q/K SBUF layout (the key trick): Z row r = 48*i + c0 decomposes as
  c0 = 16*a + b  (a<3, b<16),  r = 16*T + b with T = 3*i + a.
Heads are 1024 = 64*16 tokens, so head h's tokens t = 16*u + b map to a
CONTIGUOUS T-run [64h, 64h+64) x all b.  Storing QK as [d, b, T] makes:
  - phase-1A scatter writes CONTIGUOUS (dest addr = 2B*(3i+a), a innermost),
    instead of 96B-strided (which cost 93us ACT + 107us DVE in the old
    kernel);
  - Q/K windows rectangular APs [d; b:16; T-run].
The K/Q token enumeration becomes (b-major, u-minor); V loads and the final
output DMA apply the same permutation, so the math is unchanged.

Phase 1A computes Zt = w_qkv @ x^T in groups of 3 e-blocks (eb = m, m+8,
m+16 share the same b pair), psum [128, 3, 512], so one copy per (m, half)
covers a full contiguous T-range.  Phase 1B (V region, tokens 640:1024)
is the classic orientation, stored packed into a DRAM v_buf and read back
per-head (with the (b,v) partition permutation) during phase 2.

Matmul datapath is bf16 (1 cyc/row on the PE); accumulation fp32 in PSUM.
"""
import numpy as np
import ml_dtypes

import concourse.bass as bass
import concourse.mybir as mybir
from concourse import bacc
from concourse.tile import TileContext
from concourse.bass_utils import run_bass_kernel_spmd

N_CORES = 8
N = 1024          # tokens
DIM = 1024
E3 = 3 * DIM      # qkv projection width
H = 16            # heads
HD = 64           # head dim
SCALE = HD ** -0.5
TSPAN = 2049      # T = 3*i + a, i < 683 -> T in [0, 2049)
VROWS = 48 * 342  # 16416: V Z-rows (16384) plus 32 slack

F32 = mybir.dt.float32
BF = mybir.dt.bfloat16
FT = mybir.ActivationFunctionType


def build():
    nc = bacc.Bacc("TRN2", target_bir_lowering=False, num_devices=N_CORES)
    xt = nc.declare_dram_parameter("xt", [DIM, N], BF, isOutput=False)
    wqkvt = nc.declare_dram_parameter("wqkvt", [DIM, E3], BF, isOutput=False)
    woutt = nc.declare_dram_parameter("woutt", [DIM, DIM], BF, isOutput=False)
    bias = nc.declare_dram_parameter("bias", [1, DIM], F32, isOutput=False)
    outp = nc.declare_dram_parameter("out", [N, DIM], F32, isOutput=True)

    with TileContext(nc) as tc:
        with tc.tile_pool(name="dram", bufs=1, space="DRAM") as dpool, \
             tc.tile_pool(name="singles", bufs=1) as singles:
            # packed V buffer: flat row q = Z row 32768+q = v of head q//1024
            vbuf = dpool.tile([VROWS, HD], BF)
            vb3 = vbuf.rearrange("(a c) d -> a c d", c=48)   # (342, 48, 64)

            oT = singles.tile([128, 8, N], BF)    # [64*(h%2)+dd, h//2, i]
            biasrep = singles.tile([128, DIM], F32)
            ones_f = singles.tile([128, 8, HD], F32)
            vh0 = singles.tile([128, 8, 2 * HD], BF)
            vh1 = singles.tile([128, 8, 2 * HD], BF)
            vh2 = singles.tile([128, 8, 2 * HD], BF)
            vh3 = singles.tile([128, 8, 2 * HD], BF)
            vhs = [vh0, vh1, vh2, vh3]

            with tc.tile_pool(name="qk", bufs=1) as qkpool:
                # QKb[d, T, b] = Z[16*T + b, d]  (Q: T<1024, K: 1024<=T<2048)
                # b innermost => head windows are contiguous, token order
                # col = 16*(T-T0) + b = t - t0 (identity).
                QKb = qkpool.tile([64, TSPAN, 16], BF)
                # scatter view: T = 3*i + a, b = 2*b2 + dl
                QKs = QKb.rearrange("p (i a) (b2 dl) -> p i a b2 dl",
                                    a=3, dl=2)

                with tc.tile_pool(name="p1", bufs=1) as p1:
                    XT = p1.tile([128, 8, N], BF)
                    # WT[:, kt, a, e'] = wqkvt rows kt*128.., col a*1024+e'
                    WT = p1.tile([128, 8, 3, N], BF)
                    for kt in range(8):
                        nc.sync.dma_start(
                            out=XT[:, kt, :], in_=xt[kt * 128:(kt + 1) * 128, :])
                    # wave 1: the m<2 e-block columns of every a region, so
                    # the first psum groups are not gated on the whole load.
                    for kt in range(8):
                        for a in range(3):
                            nc.sync.dma_start(
                                out=WT[:, kt, a, 0:256],
                                in_=wqkvt[kt * 128:(kt + 1) * 128,
                                          a * N:a * N + 256])
                    for kt in range(8):
                        for a in range(3):
                            nc.sync.dma_start(
                                out=WT[:, kt, a, 256:N],
                                in_=wqkvt[kt * 128:(kt + 1) * 128,
                                          a * N + 256:(a + 1) * N])
                    nc.sync.dma_start(out=biasrep,
                                      in_=bias[:].to_broadcast((128, DIM)))
                    # [V | ones*64] stationaries for the PV matmul; ones half
                    # replicates the softmax denominator on out rows 64-127.
                    nc.vector.memset(ones_f, 1.0)
                    for v in vhs:
                        nc.vector.tensor_copy(v[:, :, HD:2 * HD], ones_f)
                        nc.vector.tensor_copy(v[:, :, 0:HD], ones_f)

                    # ---------- phase 1A: Zt = w_qkv @ x^T -> QKb ----------
                    # Group 4 e-blocks sharing one (a, m-half): eb = 8a+4mh+mm
                    # covers b = 2*(4mh+mm)+dl at fixed a, so one copy per
                    # (group, dl) writes a [i, b4] block of the [d, T, b]
                    # layout (runs of 4 bf16 at 4B step, 96B per T-triplet)
                    # instead of the old 96B-strided single elements.
                    with tc.tile_pool(name="psA", bufs=2, space="PSUM") as psA:
                        for tb in range(2):           # token halves of 384
                            i_lo = tb * 384
                            i_hi = min((tb + 1) * 384, 683)
                            cnt = i_hi - i_lo
                            for a in range(3):
                                for mh in range(2):
                                    ps4 = psA.tile([128, 4, 512], F32)
                                    for mm in range(4):
                                        m = 4 * mh + mm
                                        for kt in range(8):
                                            nc.tensor.matmul(
                                                ps4[:, mm, 0:cnt],
                                                lhsT=WT[:, kt, a,
                                                        m * 128:(m + 1) * 128],
                                                rhs=XT[:, kt, i_lo:i_hi],
                                                start=(kt == 0), stop=(kt == 7))
                                    for dl in range(2):  # c0 parity
                                        src = ps4[64 * dl:64 * dl + 64,
                                                  :, 0:cnt]
                                        src = src.rearrange("p m i -> p i m")
                                        dst = QKs[0:64, i_lo:i_hi, a,
                                                  4 * mh:4 * mh + 4, dl]
                                        if dl == 0:
                                            nc.scalar.copy(dst, src)
                                        else:
                                            nc.vector.tensor_copy(dst, src)

                    def qt_sl(h, ic):
                        w = QKb[0:64, 64 * h + 32 * ic:64 * h + 32 * ic + 32, :]
                        return w.rearrange("p T b -> p (T b)")

                    def kt_sl(h, jt):
                        w = QKb[0:64, 1024 + 64 * h + 8 * jt:
                                1024 + 64 * h + 8 * jt + 8, :]
                        return w.rearrange("p T b -> p (T b)")

                    with tc.tile_pool(name="p3", bufs=1) as p3:
                        WOT = p3.tile([128, 8, DIM], BF)
                        nc.sync.dma_start(
                            out=WOT, in_=woutt[:].rearrange("(a p) e -> p a e", p=128))

                        # ---------- phase 2 pools (sps coexists with psB) -----
                        with tc.tile_pool(name="pt", bufs=9) as ptpool, \
                             tc.tile_pool(name="rz", bufs=2) as rzpool, \
                             tc.tile_pool(name="sps", bufs=2, space="PSUM") as spsum:
                            steps = [(h, jt) for h in range(H) for jt in range(8)]

                            def load_v(h):
                                nc.sync.dma_start(
                                    out=vhs[h % 4][:, :, 0:HD],
                                    in_=vbuf[h * N:(h + 1) * N, :].rearrange(
                                        "(t p) d -> p t d", p=128))

                            def produce(h, jt):
                                sps = spsum.tile([128, 2, 512], F32, tag="sps")
                                warm = vhs[(h + 2) % 4]
                                # HAM warm-keeper; overwritten by the real scores
                                # matmul (start=True).
                                nc.tensor.matmul(
                                    sps[0:128, 0, 0:128],
                                    lhsT=warm[:, 0, :], rhs=warm[:, 0, :],
                                    start=True, stop=True)
                                for ic in range(2):
                                    nc.tensor.matmul(
                                        sps[:, ic, :],
                                        lhsT=kt_sl(h, jt),
                                        rhs=qt_sl(h, ic),
                                        start=True, stop=True)
                                pt = ptpool.tile([128, 2, 512], BF, tag="pt")
                                nc.scalar.activation(pt, sps, FT.Exp, scale=SCALE)
                                return pt

                            # ------ phase 1B: V region (tokens 640:1024) ------
                            # Head 0's produce steps are emitted between B-it6
                            # and B-it7 so the ACT exp stream starts ~10us
                            # before the PE finishes phase 1.
                            def emit_b_tile(bit, ec, pBpool, psBpool):
                                ps = psBpool.tile([128, 512], F32)
                                for kt in range(8):
                                    nc.tensor.matmul(
                                        ps,
                                        lhsT=XT[:, kt, bit * 128:(bit + 1) * 128],
                                        rhs=WT[:, kt, (ec * 512) // N,
                                               (ec * 512) % N:
                                               (ec * 512) % N + 512],
                                        start=(kt == 0), stop=(kt == 7))
                                st = pBpool.tile([128, 8, HD], BF)
                                nc.vector.tensor_copy(
                                    st, ps.rearrange("p (b d) -> p b d", d=HD))
                                # V rows: q = 48*T + c0 - 32768
                                #   c0 <= 31: q = 48*(T-683)+(c0+16)
                                #   c0 >= 32: q = 48*(T-682)+(c0-32)
                                if ec <= 3:
                                    plo = 43 if bit == 5 else 0
                                    nc.sync.dma_start(
                                        out=vb3[bit * 128 + plo - 683:
                                                (bit + 1) * 128 - 683,
                                                ec * 8 + 16: ec * 8 + 24, :],
                                        in_=st[plo:128, :, :])
                                else:
                                    plo = 42 if bit == 5 else 0
                                    nc.sync.dma_start(
                                        out=vb3[bit * 128 + plo - 682:
                                                (bit + 1) * 128 - 682,
                                                (ec - 4) * 8: (ec - 4) * 8 + 8, :],
                                        in_=st[plo:128, :, :])

                            pre_pts = []
                            with tc.tile_pool(name="pB", bufs=6) as pB, \
                                 tc.tile_pool(name="psB", bufs=4,
                                              space="PSUM") as psB:
                                for bit in (5, 6, 7):
                                    if bit == 7:
                                        # pre-produce 2 steps: ACT starts its
                                        # exp stream while B-it7 runs on PE.
                                        pre_pts = [produce(0, jt)
                                                   for jt in range(2)]
                                    for ec in range(6):
                                        emit_b_tile(bit, ec, pB, psB)
                                    if bit == 5:
                                        # V heads 0-3 live entirely in it5's
                                        # rows: preload them now.
                                        for hv in range(4):
                                            load_v(hv)

                            # ---------- phase 2: consume + produce-ahead ------
                            with tc.tile_pool(name="ops", bufs=1,
                                              space="PSUM") as opsum:
                                pts = list(pre_pts)
                                ops = None
                                for s, (h, jt) in enumerate(steps):
                                    po, hf = 64 * (h % 2), h // 2
                                    if jt == 0:
                                        pn = h % 2
                                        ops0 = opsum.tile([128, 512], F32,
                                                          tag=f"ops{2 * pn}")
                                        ops1 = opsum.tile([128, 512], F32,
                                                          tag=f"ops{2 * pn + 1}")
                                        ops = (ops0, ops1)
                                    pt_cur = pts.pop(0)
                                    if s + 2 < len(steps):
                                        pts.append(produce(*steps[s + 2]))
                                    for ic in range(2):
                                        nc.tensor.matmul(
                                            ops[ic],
                                            lhsT=vhs[h % 4][:, jt, :],
                                            rhs=pt_cur[:, ic, :],
                                            start=(jt == 0), stop=(jt == 7),
                                            skip_group_check=True)
                                    if jt == 7 and h + 4 < H:
                                        load_v(h + 4)
                                    if jt == 7:
                                        for ic in range(2):
                                            # custom-DVE reciprocal can't read
                                            # PSUM; stage through SBUF.
                                            zst = rzpool.tile([64, 512], F32,
                                                              tag="zst")
                                            nc.vector.tensor_copy(
                                                zst, ops[ic][64:128, :])
                                            rzs = rzpool.tile([64, 512], F32,
                                                              tag="rzs")
                                            nc.vector.reciprocal_approx_fast(
                                                rzs, zst)
                                            nc.vector.tensor_mul(
                                                oT[po:po + 64, hf,
                                                   ic * 512:(ic + 1) * 512],
                                                ops[ic][0:64, :], rzs)

                        # ---------- phase 3: out = oT.T @ w_out^T + b ----------
                        with tc.tile_pool(name="p3st", bufs=4) as p3st, \
                             tc.tile_pool(name="ps3", bufs=4, space="PSUM") as ps3p:
                            for it in range(8):
                                for ec in range(2):
                                    rps = ps3p.tile([128, 512], F32)
                                    for ct in range(8):
                                        nc.tensor.matmul(
                                            rps,
                                            lhsT=oT[:, ct, it * 128:(it + 1) * 128],
                                            rhs=WOT[:, ct, ec * 512:(ec + 1) * 512],
                                            start=(ct == 0), stop=(ct == 7))
                                    ost = p3st.tile([128, 512], F32)
                                    nc.vector.tensor_add(
                                        ost, rps, biasrep[:, ec * 512:(ec + 1) * 512])
                                    nc.sync.dma_start(
                                        out=outp[it * 128:(it + 1) * 128,
                                                 ec * 512:(ec + 1) * 512],
                                        in_=ost)

    nc.finalize()
    return nc


_CACHE = {}


def _get_nc():
    if "nc" not in _CACHE:
        _CACHE["nc"] = build()
    return _CACHE["nc"]


def make_in_maps(x, w_qkv, w_out, b_out):
    bf = ml_dtypes.bfloat16
    wqkvt = np.ascontiguousarray(np.asarray(w_qkv, dtype=np.float32).T).astype(bf)
    woutt = np.ascontiguousarray(np.asarray(w_out, dtype=np.float32).T).astype(bf)
    bias = np.ascontiguousarray(np.asarray(b_out, dtype=np.float32).reshape(1, DIM))
    x = np.asarray(x, dtype=np.float32)
    return [
        {
            "xt": np.ascontiguousarray(x[b].T).astype(bf),
            "wqkvt": wqkvt,
            "woutt": woutt,
            "bias": bias,
        }
        for b in range(N_CORES)
    ]


def kernel(x, w_qkv, w_out, b_out):
    nc = _get_nc()
    in_maps = make_in_maps(x, w_qkv, w_out, b_out)
    res = run_bass_kernel_spmd(nc, in_maps, core_ids=list(range(N_CORES)))
    return np.stack(
        [res.results[b]["out"] for b in range(N_CORES)], axis=0
    ).astype(np.float32)
